# revision 17
# baseline (speedup 1.0000x reference)
"""Trainium2 Bass kernel for nn_Loss_39341900431615.

Reference semantics (B,C,H,W = 16,128,128,128; only tensor[0] is read):
    idx = argmax(tensor[0,0].reshape(-1))        # row-major first max
    x0, y0 = idx // W, idx % W
    wgt[j,k] = (x0-j)^2 + (y0-k)^2               # [H,W] = [128,128]
    out[w] = sum_{j,k} wgt[j,k] * tensor[0,j,k,w]  # [W] = [128]

Sharding: the j axis (channel dim of tensor[0]) is split across 8 cores,
16 j-planes each (1 MB/core). The [128,128] argmax map tensor[0,0] is
replicated to every core (packed in the const blob) and each core
computes the argmax redundantly. Each core emits a [128] partial; the
host sums the 8 partials.

Layout: the per-core slice rides as a flat [128, 2048] tensor — SBUF
partition p holds source plane (j = p//8, k in [16*(p%8), 16*(p%8)+16))
as 8 KB contiguous rows (full-bandwidth DMA). The weighted reduction
V[p, w] = sum_klo wgt2[p, klo] * st[p, klo, w] runs as a 16-step
ping-pong scalar_tensor_tensor chain on the DVE (cheaper than 16 PE
matmuls: fp32 matmuls cost two LDWEIGHTS+MATMUL passes each). A single
PE matmul ones_col.T @ V then reduces partitions, producing the result
as a [1, 128] contiguous row so the output DMA is one 512 B descriptor
(a [128, 1] column DMA costs ~6 us in per-descriptor overhead).

Per-core device program:
  1. DMA in: blob [128, 274] (map | identity | iota | jv2 | kv2),
     slice [128, 2048].
  2. Row max + row argmax of the map via DVE max_with_indices.
  3. PE-transposes (against the identity): iota row, rowmax (f32),
     rowargmax (uint32, cast on the PSUM->SBUF copy). max_with_indices
     on the transposed rowmax gives gmax and x0 (first occurrence ==
     row-major semantics); y0 = sum_r (r==x0)*rowargmax[r] straight
     into xs via accum_out.
  4. Broadcast (x0,y0) to all partitions with a K=1 matmul against ones.
  5. wgt2[p,klo] = (jv2[p]-x0)^2 + (kv2[p,klo]-y0)^2 via DVE ops.
  6. 16-step DVE chain -> V; one PE matmul -> psum [1, 128].
  7. Copy PSUM -> SBUF row, DMA out [1, 128].
"""

import sys

for _p in ("/opt/trn_rl_repo", "/opt/pypackages"):
    if _p not in sys.path:
        sys.path.insert(0, _p)

import numpy as np

import concourse.bass as bass
from concourse import bacc
import concourse.tile as tile
from concourse import mybir
from concourse.bass_utils import run_bass_kernel_spmd

B, C, H, W = 16, 128, 128, 128
NCORES = 8
JPER = C // NCORES   # 16 j-planes per core
KLO = 16             # inner contraction steps (k per partition)
KHI = 8              # k groups per partition dim
F32 = mybir.dt.float32

# const blob column layout (the argmax map rides separately so it
# lands first and the argmax chain starts as early as possible)
COL_ID = 0             # [128, 128] identity
COL_IOTA = 128         # [128, 1] partition index 0..127
COL_JV = 129           # [128, 1] j(p) = jlo + p//8
COL_KV = 130           # [128, 16] k(p, klo) = (p%8)*16 + klo
NCOLS = 146

_CACHE = {}


def _build_bass():
    nc = bacc.Bacc("TRN2", target_bir_lowering=False, debug=False,
                   num_devices=NCORES, enable_partition_id=False)

    tmap_d = nc.dram_tensor("tmap", [H, W], F32, kind="ExternalInput")
    blob_d = nc.dram_tensor("blob", [128, NCOLS], F32, kind="ExternalInput")
    tslice = nc.dram_tensor("tslice", [128, KLO * W], F32,
                            kind="ExternalInput")
    outd = nc.dram_tensor("out", [1, W], F32, kind="ExternalOutput")

    with tile.TileContext(nc) as tc:
        with (
            tc.tile_pool(name="main", bufs=1) as pool,
            tc.tile_pool(name="psum", bufs=1, space="PSUM") as psum_pool,
        ):
            sm = pool.tile([H, W], F32)            # argmax map [x, y]
            blob = pool.tile([128, NCOLS], F32)
            st = pool.tile([128, KLO, W], F32)     # [(j,khi), klo, w]

            # st via SWDGE: issues from the (idle) gpsimd engine while the
            # sync engine issues the small map/const DMAs, so the big
            # transfer starts as early as possible.
            nc.gpsimd.dma_start(
                out=st[:, :, :],
                in_=tslice.ap().rearrange("p (a b) -> p a b", a=KLO))
            nc.sync.dma_start(out=sm[:, :], in_=tmap_d[:, :])
            nc.sync.dma_start(out=blob[:, :], in_=blob_d[:, :])

            sid = blob[:, COL_ID:COL_ID + 128]     # identity
            si = blob[:, COL_IOTA:COL_IOTA + 1]    # partition index
            jv = blob[:, COL_JV:COL_JV + 1]        # j(p)
            kv = blob[:, COL_KV:COL_KV + KLO]      # k(p, klo)

            # --- per-row max and argmax of the map ---
            vmax8 = pool.tile([128, 8], F32)
            vidx8 = pool.tile([128, 8], mybir.dt.uint32)
            nc.vector.max_with_indices(vmax8, vidx8, sm)

            vidxf = pool.tile([128, 1], F32)       # rowargmax as f32
            nc.vector.tensor_copy(vidxf, vidx8[:, 0:1])

            # --- PE transposes (iota first: its only dep is the blob DMA)
            iotaT_ps = psum_pool.tile([1, 128], F32)
            nc.tensor.transpose(iotaT_ps[:, :], si, sid)
            vmaxT_ps = psum_pool.tile([1, 128], F32)
            nc.tensor.transpose(vmaxT_ps[:, :], vmax8[:, 0:1], sid)
            vidxT_ps = psum_pool.tile([1, 128], F32)
            nc.tensor.transpose(vidxT_ps[:, :], vidxf[:, :], sid)

            iotaw = pool.tile([1, 128], F32)
            nc.vector.tensor_copy(iotaw, iotaT_ps[:, :])

            # global max over rows: value + first row index (= x0)
            # (DVE reads the transposed rowmax straight from PSUM)
            gv8 = pool.tile([1, 8], F32)
            gi8 = pool.tile([1, 8], mybir.dt.uint32)
            nc.vector.max_with_indices(gv8, gi8, vmaxT_ps[:, :])

            xs = pool.tile([1, 2], F32)            # (x0, y0) on partition 0
            nc.vector.tensor_copy(xs[:, 0:1], gi8[:, 0:1])

            # y0 = sum_r (r == x0) * rowargmax[r], straight into xs[:,1]
            ymask = pool.tile([1, 128], F32)
            nc.vector.scalar_tensor_tensor(
                ymask, in0=iotaw[:, :], scalar=xs[:, 0:1],
                in1=vidxT_ps[:, :],
                op0=mybir.AluOpType.is_equal, op1=mybir.AluOpType.mult,
                accum_out=xs[:, 1:2],
            )

            # broadcast (x0, y0) to all partitions via K=1 matmul with
            # ones; bf16 operands (x0,y0 are small ints, exact in bf16)
            # make it a single-pass matmul instead of fp32's two passes.
            BF16 = mybir.dt.bfloat16
            xsb = pool.tile([1, 2], BF16)
            nc.vector.tensor_copy(xsb, xs[:, :])
            ones = pool.tile([1, 128], BF16)
            nc.vector.memset(ones, 1.0)
            onescol = pool.tile([128, 1], F32)
            nc.vector.memset(onescol, 1.0)
            xy_ps = psum_pool.tile([128, 2], F32)
            nc.tensor.matmul(xy_ps[:, :], ones[:, :], xsb[:, :],
                             start=True, stop=True)
            xy = pool.tile([128, 2], F32)
            nc.vector.tensor_copy(xy, xy_ps[:, :])

            # --- wgt2[p, klo] = (jv[p]-x0)^2 + (kv[p,klo]-y0)^2 ---
            d1 = pool.tile([128, 1], F32)
            nc.vector.tensor_scalar(d1, jv, xy[:, 0:1], None,
                                    op0=mybir.AluOpType.subtract)
            sq1 = pool.tile([128, 1], F32)
            nc.vector.tensor_tensor(sq1, d1, d1, op=mybir.AluOpType.mult)

            d2 = pool.tile([128, KLO], F32)
            nc.vector.tensor_scalar(d2, kv, xy[:, 1:2], None,
                                    op0=mybir.AluOpType.subtract)
            sq2 = pool.tile([128, KLO], F32)
            nc.vector.tensor_tensor(sq2, d2, d2, op=mybir.AluOpType.mult)

            wgt = pool.tile([128, KLO], F32)
            nc.vector.tensor_scalar(wgt, sq2, sq1[:, 0:1], None,
                                    op0=mybir.AluOpType.add)

            # --- split reduction: klo 0..KD-1 on the DVE chain, the rest
            # as accumulating PE column matmuls (both engines run in
            # parallel once wgt is ready) ---
            KD = 8  # klo steps on DVE; KLO-KD on PE

            # DVE half: V[p, w] = sum_{klo<KD} st[p, klo, :] * wgt[p, klo]
            va = pool.tile([128, W], F32)
            vb = pool.tile([128, W], F32)
            nc.vector.tensor_scalar(va, st[:, 0, :], wgt[:, 0:1], None,
                                    op0=mybir.AluOpType.mult)
            cur, nxt = va, vb
            last_chain = None
            for klo in range(1, KD):
                last_chain = nc.vector.scalar_tensor_tensor(
                    nxt, in0=st[:, klo, :], scalar=wgt[:, klo:klo + 1],
                    in1=cur,
                    op0=mybir.AluOpType.mult, op1=mybir.AluOpType.add)
                cur, nxt = nxt, cur

            # PE half: col[w, 1] = sum_{klo>=KD} st[:, klo, :].T @ wgt col
            col_ps = psum_pool.tile([128, 1], F32)
            for i, klo in enumerate(range(KD, KLO)):
                nc.tensor.matmul(col_ps[:, :], st[:, klo, :],
                                 wgt[:, klo:klo + 1],
                                 start=(i == 0), stop=(klo == KLO - 1))

            # Move the PE half's column to SBUF. The explicit dep keeps the
            # scheduler from placing this copy (which waits on PE) before
            # the chain in DVE program order, which would serialize the
            # two halves.
            colv = pool.tile([128, 1], F32)
            ci = nc.vector.tensor_copy(colv, col_ps[:, :])
            if last_chain is not None:
                tile.add_dep_helper(ci.ins, last_chain.ins, sync=False,
                                    reason="colv copy after DVE chain")

            # One PSUM accumulation group: transpose(col) then add
            # ones_col.T @ V -> row_ps = full partition-reduced result.
            row_ps = psum_pool.tile([1, W], F32)
            nc.tensor.matmul(row_ps[:, :], onescol[:, :], cur[:, :],
                             start=True, stop=False)
            nc.tensor.matmul(row_ps[:, :], colv[:, :], sid,
                             is_transpose=True, start=False, stop=True)

            outv = pool.tile([1, W], F32)
            nc.vector.tensor_copy(outv, row_ps[:, :])
            nc.sync.dma_start(out=outd[:, :], in_=outv[:, :])

    return nc


def _get_bass():
    if "nc" not in _CACHE:
        nc = _build_bass()
        nc.finalize()
        _CACHE["nc"] = nc
    return _CACHE["nc"]


def _make_blob(jlo):
    blob = np.zeros((128, NCOLS), dtype=np.float32)
    p = np.arange(128)
    blob[:, COL_ID:COL_ID + 128] = np.eye(128, dtype=np.float32)
    blob[:, COL_IOTA] = p
    blob[:, COL_JV] = jlo + p // KHI
    blob[:, COL_KV:COL_KV + KLO] = ((p % KHI) * KLO)[:, None] + np.arange(KLO)
    return blob


def _make_in_maps(tensor):
    t0 = np.ascontiguousarray(tensor[0], dtype=np.float32)  # [C,H,W]
    in_maps = []
    tmap = np.ascontiguousarray(t0[0])
    for c in range(NCORES):
        jlo = c * JPER
        in_maps.append({
            "tmap": tmap,
            "blob": _make_blob(jlo),
            "tslice": np.ascontiguousarray(
                t0[jlo:jlo + JPER].reshape(128, KLO * W)),
        })
    return in_maps


def kernel(tensor):
    nc = _get_bass()
    res = run_bass_kernel_spmd(nc, _make_in_maps(tensor),
                               core_ids=list(range(NCORES)))
    partials = np.stack([r["out"].reshape(W) for r in res.results])
    return partials.astype(np.float64).sum(axis=0).astype(np.float32)
